# revision 16
# baseline (speedup 1.0000x reference)
"""BSRBF-KAN layer forward on 8 Trainium2 cores (Bass/Tile).

Math (per token t, output o):
    xn = LayerNorm(x) * g + b
    out[t,o] = sum_d relu(xn[t,d]) * Wb[o,d]
             + sum_{d,j} (B_j(xn[t,d]) + G_j(xn[t,d])) * Ws[o, d*8+j]

B_j: cardinal cubic B-spline on uniform knots (h=0.6, centers c_j=-2.1+0.6j):
    B_j(x) = [relu(2h-|x-c_j|)^3 - 4*relu(h-|x-c_j|)^3] / (6h^3)
computed by a fused custom DVE op (2 instructions per j):
    OP(x; s0,s1,imm2, src1) = max(min(s0-x, x-s1), 0)^3 * imm2 + src1
G_j: Gaussians exp(-((x-r_j)/D)^2), r_j uniform; anchors j in {0,4} via
ACT Square+Exp, the rest by the recurrence G_j = (G_{j-1}*c_j)*exp(d*x)
(one scalar_tensor_tensor each; algebraically exact).

The 9 feature channels (8 bsrbf + relu) feed a K=4608 fp32r matmul
(tokens as lhsT M-dim, 512 outputs as rhs N-dim), PSUM-accumulated.
Data-parallel: tokens sharded 8 ways, weights replicated.
"""

import numpy as np

# ---------------------------------------------------------------- constants
B, S, D, O = 4, 4096, 512, 512
TOKENS = B * S
CORES = 8
TPC = TOKENS // CORES          # tokens per core (2048)
NB = 8                         # basis funcs per input dim
H = 0.6                        # knot spacing
CJ = [-2.1 + 0.6 * j for j in range(NB)]   # spline centers
DELTA = 3.0 / 7.0              # rbf denom
RJ = [-1.5 + j * (3.0 / 7.0) for j in range(NB)]  # rbf centers
DLT = 2.0 * (3.0 / 7.0) / DELTA**2   # = 14/3, exponent scale of Q
LN_EPS = 1e-5
CUBE_SCALE = 1.0 / (6.0 * H**3)

# chain fold: channel_j = F_j + B_j/RHO[j], F_j = F_{j-1}*exp(DLT*x),
# weights scaled by RHO[j].  RHO = prod of per-step gaussian ratios.
_CC = {j: float(np.exp(-(3.0 / 7.0) * (RJ[j] + RJ[j - 1]) / DELTA**2))
       for j in (1, 2, 3, 5, 6, 7)}
RHO = [1.0] * NB
for _j in (1, 2, 3):
    RHO[_j] = RHO[_j - 1] * _CC[_j]
RHO[4] = 1.0
for _j in (5, 6, 7):
    RHO[_j] = RHO[_j - 1] * _CC[_j]

BLK = 512                      # tokens per processing block
NBLK = TPC // BLK              # 4 blocks per core
QCH = D // 128                 # 4 d-chunks
NCH = NB + 1                   # 9 matmul channels per d-chunk
KT = QCH * NCH                 # 36 k-tiles

_BUILT = {}


# ------------------------------------------------------- custom DVE op
def _get_custom_op():
    """Register (idempotently) the fused spline-side op:
        out = max(min(s0 - in0, in0 - s1), 0)^3 * imm2 + in1
    """
    import concourse.dve_ops as dve_ops
    from concourse.dve_ops import DveOp
    from concourse.dve_spec import (
        Spec, Src0, Src1, C0, C1, C2, Zero, maxx, minn, sq, lower,
    )
    from concourse.dve_uop import DveOpSpec
    from concourse.dve_table_gen import dve_ver_for

    NAME = "BSPLINE_SIDE_ANT"
    for op in dve_ops.OPS:
        if op.name == NAME:
            return op

    hi = C0 - Src0
    lo = Src0 - C1
    m = maxx(minn(hi, lo), Zero)
    body = sq(m) * m * C2 + Src1

    def _ref(in0, in1, s0, s1, imm2):
        return (
            np.maximum(np.minimum(s0 - in0, in0 - s1), 0.0) ** 3 * imm2 + in1
        ).astype(np.float32)

    spec = Spec(body=body, reference=_ref)

    row = max(dve_ops._SUB_OPCODE_FOR_NAME.values()) + 1
    assert row < 0x20
    dve_ops._SUB_OPCODE_FOR_NAME[NAME] = row

    shas = {}
    for ver in ("v3", "v4"):
        try:
            uops = lower(spec, ver=ver)
            shas[ver] = DveOpSpec(name=NAME, opcode=row, uops=uops,
                                  rd1_en=True).sha(ver)
        except Exception:
            pass
    op = DveOp(NAME, spec, subdim=False, uops_sha=shas)
    dve_ops.OPS.append(op)
    dve_ops.CUSTOM_DVE_SPECS[NAME] = spec
    return op


# ------------------------------------------------------- bass program
def _build_program(matmul_dt_name="float32r"):
    import concourse.bass as bass
    import concourse.bacc as bacc
    import concourse.mybir as mybir
    import concourse.tile as tile
    from contextlib import ExitStack

    OPC = _get_custom_op()
    f32 = mybir.dt.float32
    mm_dt = getattr(mybir.dt, matmul_dt_name)
    AF = mybir.ActivationFunctionType
    ALU = mybir.AluOpType

    nc = bacc.Bacc("TRN2", target_bir_lowering=False, debug=False)
    xs = nc.declare_dram_parameter("xs", [TPC, D], f32, isOutput=False)
    wcat = nc.declare_dram_parameter("wcat", [KT * 128, O], mm_dt, isOutput=False)
    gmt = nc.declare_dram_parameter("gmt", [128, QCH], f32, isOutput=False)
    bet = nc.declare_dram_parameter("bet", [128, QCH], f32, isOutput=False)
    idn = nc.declare_dram_parameter("idn", [128, 128], f32, isOutput=False)
    out = nc.declare_dram_parameter("out", [TPC, O], f32, isOutput=True)

    def _register_const(val):
        key = (f32, float(val))
        if key not in nc.const_aps.aps:
            t = nc.alloc_sbuf_tensor(
                f"constf32_{len(nc.const_aps.aps)}", [128, 1], f32)
            nc.gpsimd.memset(t.ap(), float(val))
            nc.const_aps.aps[key] = t.ap()
    _register_const(LN_EPS)
    for j in (0, 4):
        _register_const(-RJ[j] / DELTA)
    nc.all_engine_barrier()

    with ExitStack() as ctx:
        tc = ctx.enter_context(tile.TileContext(nc))

        const_pool = ctx.enter_context(tc.tile_pool(name="const", bufs=1))
        w_pool = ctx.enter_context(tc.tile_pool(name="wts", bufs=1))
        x_pool = ctx.enter_context(tc.tile_pool(name="x", bufs=6))
        stat_pool = ctx.enter_context(tc.tile_pool(name="stat", bufs=10))
        xn_pool = ctx.enter_context(tc.tile_pool(name="xn", bufs=4))
        xnt_pool = ctx.enter_context(tc.tile_pool(name="xnt", bufs=5))
        rbf_pool = ctx.enter_context(tc.tile_pool(name="rbf", bufs=4))
        q_pool = ctx.enter_context(tc.tile_pool(name="q", bufs=2))
        t1_pool = ctx.enter_context(tc.tile_pool(name="t1", bufs=2))
        feat_pool = ctx.enter_context(tc.tile_pool(name="feat", bufs=4))
        relu_pool = ctx.enter_context(tc.tile_pool(name="relu", bufs=2))
        osb_pool = ctx.enter_context(tc.tile_pool(name="osb", bufs=4))
        tp_psum = ctx.enter_context(tc.tile_pool(name="tpp", bufs=2, space="PSUM"))
        out_psum = ctx.enter_context(tc.tile_pool(name="opp", bufs=4, space="PSUM"))

        # --- constants / weights to SBUF
        ident = const_pool.tile([128, 128], f32, tag="ident")
        nc.sync.dma_start(ident[:], idn[:, :])
        gam = const_pool.tile([128, QCH], f32, tag="gam")
        nc.sync.dma_start(gam[:], gmt[:, :])
        bta = const_pool.tile([128, QCH], f32, tag="bta")
        nc.sync.dma_start(bta[:], bet[:, :])

        wt = []
        for kt in range(KT):
            w = w_pool.tile([128, O], mm_dt, tag=f"w{kt}")
            nc.sync.dma_start(w[:], wcat[kt * 128:(kt + 1) * 128, :])
            wt.append(w)

        for blk in range(NBLK):
            # ---- load + layernorm, 4 token-tiles of [128, D]
            xn_tiles = []
            for i in range(4):
                t0 = blk * BLK + i * 128
                xt = x_pool.tile([128, D], f32)
                nc.sync.dma_start(xt[:], xs[t0:t0 + 128, :])
                st6 = stat_pool.tile([128, 6], f32, tag="st6")
                nc.vector.bn_stats(st6[:], xt[:])
                mv = stat_pool.tile([128, 2], f32, tag="mv")
                nc.vector.bn_aggr(mv[:], st6[:])
                sd = stat_pool.tile([128, 1], f32, tag="sd")
                nc.scalar.activation(sd[:], mv[:, 1:2], AF.Sqrt, bias=LN_EPS)
                rstd = stat_pool.tile([128, 1], f32, tag="rstd")
                nc.vector.reciprocal(rstd[:], sd[:])
                xnt_ = xn_pool.tile([128, D], f32)
                nc.gpsimd.tensor_scalar(
                    xnt_[:], xt[:], mv[:, 0:1], rstd[:],
                    op0=ALU.subtract, op1=ALU.mult)
                xn_tiles.append(xnt_)

            # ---- transpose to [128d, BLK t] per d-chunk, apply gamma/beta
            xnT = []
            for q in range(QCH):
                pt = tp_psum.tile([128, BLK], f32, tag="pt",
                                  name=f"pt{blk}_{q}")
                for i in range(4):
                    nc.tensor.transpose(
                        pt[:, i * 128:(i + 1) * 128],
                        xn_tiles[i][:, q * 128:(q + 1) * 128],
                        ident[:])
                xq = xnt_pool.tile([128, BLK], f32, tag="xq",
                                   name=f"xq{blk}_{q}")
                nc.scalar.activation(
                    xq[:], pt[:], AF.Identity,
                    bias=bta[:, q:q + 1], scale=gam[:, q:q + 1])
                xnT.append(xq)

            # ---- features + matmuls per d-chunk
            po = [out_psum.tile([128, O], f32, tag="po", name=f"po{blk}_{m}")
                  for m in range(4)]
            for q in range(QCH):
                xq = xnT[q]
                qt = q_pool.tile([128, BLK], f32, tag="qt", name=f"qt{blk}_{q}")
                nc.scalar.activation(qt[:], xq[:], AF.Exp, scale=DLT)
                rl = relu_pool.tile([128, BLK], mm_dt, tag="rl",
                                    name=f"rl{blk}_{q}")
                nc.scalar.activation(rl[:], xq[:], AF.Relu)

                feats = []
                rbf_prev = None
                for j in range(NB):
                    r = rbf_pool.tile([128, BLK], f32, tag="rbf",
                                      name=f"rbf{blk}_{q}_{j}")
                    if j in (0, 4):
                        z2 = rbf_pool.tile([128, BLK], f32, tag="z2",
                                           name=f"z2{blk}_{q}_{j}")
                        nc.scalar.activation(
                            z2[:], xq[:], AF.Square,
                            bias=-RJ[j] / DELTA, scale=1.0 / DELTA)
                        nc.scalar.activation(r[:], z2[:], AF.Exp, scale=-1.0)
                    else:
                        nc.gpsimd.tensor_tensor(
                            r[:], rbf_prev[:], qt[:], op=ALU.mult)
                    rbf_prev = r
                    t1 = t1_pool.tile([128, BLK], f32, tag="t1",
                                      name=f"t1{blk}_{q}_{j}")
                    nc.vector._custom_dve(
                        OPC, out=t1[:], in0=xq[:], in1=r[:],
                        s0=CJ[j] + 2 * H, s1=CJ[j] - 2 * H,
                        imm2=CUBE_SCALE / RHO[j])
                    bs = feat_pool.tile([128, BLK], mm_dt, tag="bsrbf",
                                        name=f"bs{blk}_{q}_{j}")
                    nc.vector._custom_dve(
                        OPC, out=bs[:], in0=xq[:], in1=t1[:],
                        s0=CJ[j] + H, s1=CJ[j] - H,
                        imm2=-4.0 * CUBE_SCALE / RHO[j])
                    feats.append(bs)
                feats.append(rl)

                for ch in range(NCH):
                    f = feats[ch]
                    w = wt[q * NCH + ch]
                    for m in range(4):
                        nc.tensor.matmul(
                            po[m][:],
                            f[:, m * 128:(m + 1) * 128],
                            w[:],
                            start=(q == 0 and ch == 0),
                            stop=(q == QCH - 1 and ch == NCH - 1))

            # ---- evacuate + store
            for m in range(4):
                ot = osb_pool.tile([128, O], f32, tag="ot",
                                   name=f"ot{blk}_{m}")
                nc.scalar.copy(ot[:], po[m][:])
                t0 = blk * BLK + m * 128
                nc.sync.dma_start(out[t0:t0 + 128, :], ot[:])

    nc.compile()
    return nc


def _host_prep(x, ln_weight, ln_bias, base_weight, spline_weight):
    x = np.ascontiguousarray(np.asarray(x, dtype=np.float32)).reshape(TOKENS, D)
    ln_weight = np.asarray(ln_weight, dtype=np.float32)
    ln_bias = np.asarray(ln_bias, dtype=np.float32)
    base_weight = np.asarray(base_weight, dtype=np.float32)
    spline_weight = np.asarray(spline_weight, dtype=np.float32)

    # wcat[(q*9+ch)*128 + dl, o]
    wsp = spline_weight.reshape(O, D, NB)          # [o, d, j]
    blocks = np.empty((QCH, NCH, 128, O), dtype=np.float32)
    wsp_t = np.transpose(wsp, (1, 2, 0))            # [d, j, o]
    rho = np.asarray(RHO, dtype=np.float64)[:, None, None]
    for q in range(QCH):
        blocks[q, :NB] = (np.transpose(
            wsp_t[q * 128:(q + 1) * 128], (1, 0, 2)).astype(np.float64)
            * rho).astype(np.float32)  # [j, dl, o]
        blocks[q, NB] = base_weight.T[q * 128:(q + 1) * 128]
    wcat = np.ascontiguousarray(blocks.reshape(KT * 128, O))

    gmt = np.ascontiguousarray(ln_weight.reshape(QCH, 128).T)
    bet = np.ascontiguousarray(ln_bias.reshape(QCH, 128).T)
    idn = np.eye(128, dtype=np.float32)
    return x, wcat, gmt, bet, idn


def kernel(x, ln_weight, ln_bias, base_weight, spline_weight):
    from concourse.bass_utils import run_bass_kernel_spmd

    if "nc" not in _BUILT:
        _BUILT["nc"] = _build_program()
    nc = _BUILT["nc"]

    xf, wcat, gmt, bet, idn = _host_prep(
        x, ln_weight, ln_bias, base_weight, spline_weight)

    in_maps = []
    for c in range(CORES):
        in_maps.append({
            "xs": np.ascontiguousarray(xf[c * TPC:(c + 1) * TPC]),
            "wcat": wcat, "gmt": gmt, "bet": bet, "idn": idn,
        })
    res = run_bass_kernel_spmd(nc, in_maps, core_ids=list(range(CORES)))
    outs = [res.results[c]["out"] for c in range(CORES)]
    full = np.concatenate(outs, axis=0).reshape(B, S, O)
    return full.astype(np.float32)
